# revision 2
# baseline (speedup 1.0000x reference)
"""Trainium2 Bass kernel for nn_HcPost:

    out[b,s,n,d] = post[b,s,n] * x[b,s,d] + sum_m comb[b,s,m,n] * residual[b,s,m,d]

Strategy: per token this is a tiny K=5 contraction
    out[n,d] = sum_{m'} Caug[m',n] * Xaug[m',d]
with Xaug = [x; residual_0..3] and Caug = [post; comb_0..3].

We batch G=25 tokens into one TensorE matmul by building a block-diagonal
stationary weight matrix W[(t,m'), (t,n)] = Caug[t,m',n] (K=125, MF=100) on the
host, and streaming Xaug[(t,m'), d] as the moving operand. PSUM results
[(t,n), d] are evacuated to SBUF by VectorE/ScalarE and DMA'd out.

The kernel is HBM-bandwidth bound (~155 MB/core of fp32 traffic), so all HBM
I/O is done in bf16 (inputs downcast on host, outputs upcast on host) —
halving traffic to ~78 MB/core. PSUM accumulation stays fp32; measured
end-to-end relative error ~5e-3.

Sharding: tokens (B*S = 16384) split evenly across 8 NeuronCores (data
parallel, no cross-core communication). Tokens are padded to 2050/core so each
core runs 82 uniform groups of 25.
"""

import sys

sys.path.insert(0, "/opt/trn_rl_repo")

import ml_dtypes
import numpy as np

import concourse.bass as bass
import concourse.mybir as mybir
import concourse.tile as tile
from concourse import bacc
from concourse.bass_utils import run_bass_kernel_spmd

B, S, M, N, D = 4, 4096, 4, 4, 2048
TOK = B * S  # 16384 tokens
N_CORES = 8
G = 25  # tokens per PE group (contraction K = 5*G = 125 <= 128)
KDIM = 5 * G  # 125
MF = N * G  # 100 output partitions per group
TPC = 2050  # padded tokens per core (= 82 * 25)
NG = TPC // G  # 82 groups per core
TOKP = TPC * N_CORES  # 16400 padded tokens total
GP = 4  # groups per DMA chunk (batches DMAs to ~4 MB)
DCH = 512  # moving free-dim chunk (one PSUM bank)

BF16 = ml_dtypes.bfloat16

# Stashed BassKernelResults of the last kernel() call (for profiling in test
# harnesses via BASS_TRACE=1).
LAST_RESULTS = None
LAST_IN_MAPS = None

# Best configuration found on HW: all bulk DMAs via gpsimd SWDGE (spreads
# descriptors over all 16 SDMA engines; HWDGE pins reads to engines 0-4 /
# writes to 0-9), single-group chunks with deep double-buffering, output DMAs
# delayed by several chunks so they never head-of-line block input DMAs in the
# Pool FIFO, weight slices interleaved into the first chunks. All HBM I/O in
# bf16.
BUILD_KWARGS = dict(
    in_eng="gpsimd",
    gp=1,
    abufs=10,
    obufs=9,
    out_spart=100,
    out_delay=6,
    wsplit=8,
    weng="ginter",
    io_dtype="bfloat16",
)


def _build_program(in_eng="sync", out_eng="sync", in_split=1, out_split=1,
                   out_hpart=0, gp=GP, abufs=2, obufs=2, pbufs=8,
                   out_spart=0, out_delay=4, wsplit=1, weng="sync",
                   io_dtype="bfloat16", copy_banks=1):
    """Build the SPMD Bass program.

    in_eng/out_eng: comma-separated engine cycle for input/output DMAs —
    elements from {sync, scalar, gpsimd}. Successive chunks rotate through
    the cycle. in_split/out_split: issue each chunk's DMA as this many
    instructions (split along the partition dim). out_hpart: if >0, rows
    [0, out_hpart) of each output chunk go via sync HWDGE and the rest via
    gpsimd SWDGE (overrides out_eng). io_dtype: dtype of all HBM tensors and
    SBUF tiles (PSUM accumulation is always fp32).
    """
    f32 = mybir.dt.float32
    iodt = getattr(mybir.dt, io_dtype)
    nc = bacc.Bacc(None, target_bir_lowering=False)
    xa = nc.dram_tensor("xa", [TPC * 5, D], iodt, kind="ExternalInput")
    wb = nc.dram_tensor("wb", [KDIM, NG * MF], iodt, kind="ExternalInput")
    y = nc.dram_tensor("y", [TPC * N, D], iodt, kind="ExternalOutput")

    def engines(spec):
        return [getattr(nc, e) for e in spec.split(",")]

    in_engs = engines(in_eng)
    out_engs = engines(out_eng)

    chunks = []
    g = 0
    while g < NG:
        chunks.append((g, min(gp, NG - g)))
        g += chunks[-1][1]

    # Row r = t*5 + m' of xa is one (token, m') slice; groups are 125 rows.
    xa_v = xa[:].rearrange("(G p) d -> G p d", p=KDIM)
    # Row r = t*4 + n of y; groups are 100 rows.
    y_v = y[:].rearrange("(G p) d -> G p d", p=MF)

    def split_dma(eng, dst, src, nsplit, pdim):
        if nsplit == 1:
            eng.dma_start(dst, src)
            return
        step = (pdim + nsplit - 1) // nsplit
        for s0 in range(0, pdim, step):
            s1 = min(s0 + step, pdim)
            eng.dma_start(dst[s0:s1], src[s0:s1])

    with tile.TileContext(nc) as tc:
        with (
            tc.tile_pool(name="wpool", bufs=1) as wpool,
            tc.tile_pool(name="apool", bufs=abufs) as apool,
            tc.tile_pool(name="opool", bufs=obufs) as opool,
            tc.tile_pool(name="psum", bufs=pbufs, space=bass.MemorySpace.PSUM) as psum,
        ):
            gper = (NG + wsplit - 1) // wsplit
            interleave_w = weng == "ginter"
            wt_tiles = []
            w_eng = nc.gpsimd if interleave_w else getattr(nc, weng)

            def load_w(wi):
                glo = wi * gper
                ghi = min(NG, (wi + 1) * gper)
                wtile = wpool.tile([KDIM, (ghi - glo) * MF], iodt, tag=f"w{wi}")
                w_eng.dma_start(wtile[:], wb[:, glo * MF : ghi * MF])
                wt_tiles.append(wtile)

            if not interleave_w:
                for wi in range(wsplit):
                    load_w(wi)

            def w_slice(g):
                wi, off = divmod(g, gper)
                return wt_tiles[wi][:, off * MF : (off + 1) * MF]

            k = 0
            pending = []  # delayed SWDGE output DMAs: (dst_ap, src_tile_ap)
            for ci, (gstart, cgp) in enumerate(chunks):
                a = apool.tile([KDIM, cgp, D], iodt, tag="a")
                split_dma(
                    in_engs[ci % len(in_engs)],
                    a[:],
                    xa_v[gstart : gstart + cgp].rearrange("g p d -> p g d"),
                    in_split,
                    KDIM,
                )
                if interleave_w and ci < wsplit:
                    load_w(ci)
                if out_spart > 0 and len(pending) >= out_delay:
                    dst, src = pending.pop(0)
                    nc.gpsimd.dma_start(dst, src)
                o = opool.tile([MF, cgp, D], iodt, tag="o")
                for gs in range(cgp):
                    gw = gstart + gs
                    for dcb in range(0, D // DCH, copy_banks):
                        p = psum.tile([MF, copy_banks * DCH], f32)
                        for j in range(copy_banks):
                            dc = dcb + j
                            nc.tensor.matmul(
                                p[:, j * DCH : (j + 1) * DCH],
                                lhsT=w_slice(gw),
                                rhs=a[:, gs, dc * DCH : (dc + 1) * DCH],
                                start=True,
                                stop=True,
                            )
                        dst = o[:, gs, dcb * DCH : (dcb + copy_banks) * DCH]
                        if k % 2 == 0:
                            nc.vector.tensor_copy(dst, p[:])
                        else:
                            nc.scalar.copy(dst, p[:])
                        k += 1
                y_dst = y_v[gstart : gstart + cgp].rearrange("g p d -> p g d")
                if out_spart > 0:
                    hp = MF - out_spart
                    if hp > 0:
                        nc.sync.dma_start(y_dst[:hp], o[:hp])
                    pending.append((y_dst[hp:], o[hp:]))
                elif out_hpart > 0:
                    nc.sync.dma_start(y_dst[:out_hpart], o[:out_hpart])
                    nc.gpsimd.dma_start(y_dst[out_hpart:], o[out_hpart:])
                else:
                    split_dma(
                        out_engs[ci % len(out_engs)],
                        y_dst,
                        o[:],
                        out_split,
                        MF,
                    )
            for dst, src in pending:
                nc.gpsimd.dma_start(dst, src)
    nc.compile()
    return nc


def _np_io_dtype():
    return {"bfloat16": BF16, "float16": np.float16, "float32": np.float32}[
        BUILD_KWARGS.get("io_dtype", "bfloat16")
    ]


def kernel(x, residual, post, comb):
    global LAST_RESULTS, LAST_IN_MAPS
    x = np.asarray(x, dtype=np.float32)
    residual = np.asarray(residual, dtype=np.float32)
    post = np.asarray(post, dtype=np.float32)
    comb = np.asarray(comb, dtype=np.float32)
    iodt = _np_io_dtype()

    # Host prepack: augmented data rows (token-major) and block-diagonal
    # weights, downcast to the I/O dtype. Padded tokens have zero weights ->
    # zero output rows.
    xaug = np.zeros((TOKP, 5, D), iodt)
    xaug[:TOK, 0, :] = x.reshape(TOK, D)
    xaug[:TOK, 1:, :] = residual.reshape(TOK, M, D)

    caug = np.zeros((TOKP, 5, N), np.float32)
    caug[:TOK, 0, :] = post.reshape(TOK, N)
    caug[:TOK, 1:, :] = comb.reshape(TOK, M, N)

    ngt = TOKP // G  # total groups
    wall = np.zeros((ngt, KDIM, MF), np.float32)
    t = np.arange(G)
    rows = np.broadcast_to(
        5 * t[:, None, None] + np.arange(5)[None, :, None], (G, 5, N)
    ).ravel()
    cols = np.broadcast_to(
        N * t[:, None, None] + np.arange(N)[None, None, :], (G, 5, N)
    ).ravel()
    wall[:, rows, cols] = caug.reshape(ngt, G * 5 * N)

    in_maps = []
    for c in range(N_CORES):
        xa_c = np.ascontiguousarray(xaug[c * TPC : (c + 1) * TPC].reshape(TPC * 5, D))
        wb_c = np.ascontiguousarray(
            wall[c * NG : (c + 1) * NG].transpose(1, 0, 2).reshape(KDIM, NG * MF)
        ).astype(iodt)
        in_maps.append({"xa": xa_c, "wb": wb_c})

    LAST_IN_MAPS = in_maps
    nc = _build_program(**BUILD_KWARGS)
    res = run_bass_kernel_spmd(nc, in_maps, list(range(N_CORES)))
    LAST_RESULTS = res

    y = np.concatenate(
        [res.results[c]["y"].reshape(TPC, N, D) for c in range(N_CORES)], axis=0
    )[:TOK]
    return np.ascontiguousarray(y.reshape(B, S, N, D).astype(np.float32))


# revision 10
# speedup vs baseline: 1.0316x; 1.0316x over previous
"""Trainium2 Bass kernel for nn_HcPost:

    out[b,s,n,d] = post[b,s,n] * x[b,s,d] + sum_m comb[b,s,m,n] * residual[b,s,m,d]

Strategy: per token this is a tiny K=5 contraction
    out[n,d] = sum_{m'} Caug[m',n] * Xaug[m',d]
with Xaug = [x; residual_0..3] and Caug = [post; comb_0..3].

We batch G=25 tokens into one TensorE matmul by building a block-diagonal
stationary weight matrix W[(t,m'), (t,n)] = Caug[t,m',n] (K=125, MF=100) on the
host, and streaming Xaug[(t,m'), d] as the moving operand. PSUM results
[(t,n), d] are evacuated to SBUF by VectorE/ScalarE and DMA'd out.

The kernel is HBM-bandwidth bound (~155 MB/core of fp32 traffic), so:
  - All HBM I/O is bf16 (inputs downcast on host, outputs upcast on host),
    halving traffic to ~78 MB/core. PSUM accumulation stays fp32; measured
    end-to-end relative error ~6e-3 (gate 2e-2).
  - HBM arrays are PARTITION-MAJOR ([p, group, d]) so each SDMA descriptor
    covers gp*4KB contiguous per partition instead of 4KB — descriptor
    fixed costs were ~30% of DMA busy time in the token-major layout.

Sharding: tokens (B*S = 16384) split evenly across 8 NeuronCores (data
parallel, no cross-core communication). Tokens are padded to 2050/core so each
core runs 82 uniform groups of 25.
"""

import sys

sys.path.insert(0, "/opt/trn_rl_repo")

import ml_dtypes
import numpy as np

import concourse.bass as bass
import concourse.mybir as mybir
import concourse.tile as tile
from concourse import bacc
from concourse.bass_utils import run_bass_kernel_spmd


def _ensure_ntff_hook():
    """Best-effort: register the axon NTFF profile hook so a BASS_TRACE=1 run
    can report exec_time_ns. The agent image's ``antenv`` lacks
    ``axon_hooks``; inject a minimal stand-in. No-op off-axon or on failure.
    """
    try:
        from concourse.bass_utils import axon_active

        if not axon_active():
            return
        try:
            from antenv.axon_hooks import get_axon_ntff_profile_hook  # noqa: F401

            return  # real module present
        except ImportError:
            pass
        import types

        import antenv
        from trn_agent_boot.trn_boot import _ntff_profile_via_ctypes

        mod = types.ModuleType("antenv.axon_hooks")
        mod._hook = _ntff_profile_via_ctypes("/opt/axon/libaxon_pjrt.so")
        mod.set_axon_ntff_profile_hook = lambda h: setattr(mod, "_hook", h)
        mod.get_axon_ntff_profile_hook = lambda: mod._hook
        sys.modules["antenv.axon_hooks"] = mod
        antenv.axon_hooks = mod
    except Exception:
        pass


_ensure_ntff_hook()

B, S, M, N, D = 4, 4096, 4, 4, 2048
TOK = B * S  # 16384 tokens
N_CORES = 8
G = 25  # tokens per PE group (contraction K = 5*G = 125 <= 128)
KDIM = 5 * G  # 125
MF = N * G  # 100 output partitions per group
TPC = 2050  # padded tokens per core (= 82 * 25)
NG = TPC // G  # 82 groups per core
TOKP = TPC * N_CORES  # 16400 padded tokens total
DCH = 512  # moving free-dim chunk (one PSUM bank)

BF16 = ml_dtypes.bfloat16

# Stashed BassKernelResults of the last kernel() call (for profiling in test
# harnesses via BASS_TRACE=1).
LAST_RESULTS = None
LAST_IN_MAPS = None

# Best configuration found on HW: all bulk DMAs via gpsimd SWDGE (spreads
# descriptors over all 16 SDMA engines; HWDGE pins reads to engines 0-4 /
# writes to 0-9), output DMAs delayed by several chunks so they never
# head-of-line block input DMAs in the gpsimd FIFO, weight slices interleaved
# into the first chunks. All HBM I/O bf16 + partition-major.
BUILD_KWARGS = dict(
    in_eng="gpsimd",
    gp=1,
    abufs=10,
    obufs=9,
    out_spart=100,
    out_delay=3,
    out_flush=1,
    wsplit=8,
    weng="ginter",
    io_dtype="bfloat16",
)


def _build_program(in_eng="sync", out_eng="sync", in_split=1, out_split=1,
                   out_hpart=0, gp=4, abufs=2, obufs=2, pbufs=8,
                   out_spart=0, out_delay=4, out_flush=1, wsplit=1,
                   weng="sync", io_dtype="bfloat16", copy_banks=1,
                   probe=None):
    """Build the SPMD Bass program.

    HBM layouts (per core): xa [KDIM, NG*D] partition-major (xa[p, g*D+d] is
    (token g*25+p//5, m'=p%5, d) of the augmented input), wb [KDIM, NG*MF]
    block-diagonal weights, y [MF, NG*D] partition-major outputs.

    in_eng/out_eng: comma-separated engine cycle for input/output DMAs —
    elements from {sync, scalar, gpsimd}. in_split/out_split: issue each
    chunk's DMA as this many instructions (split along the partition dim).
    out_spart: if >0, that many rows of each output chunk go via delayed
    gpsimd SWDGE (delayed by out_delay chunks), the rest via sync HWDGE.
    io_dtype: dtype of all HBM tensors and SBUF tiles (PSUM stays fp32).
    """
    f32 = mybir.dt.float32
    iodt = getattr(mybir.dt, io_dtype)
    nc = bacc.Bacc(None, target_bir_lowering=False)
    xa = nc.dram_tensor("xa", [KDIM, NG * D], iodt, kind="ExternalInput")
    wb = nc.dram_tensor("wb", [KDIM, NG * MF], iodt, kind="ExternalInput")
    y = nc.dram_tensor("y", [MF, NG * D], iodt, kind="ExternalOutput")

    def engines(spec):
        return [getattr(nc, e) for e in spec.split(",")]

    in_engs = engines(in_eng)
    out_engs = engines(out_eng)

    chunks = []
    g = 0
    while g < NG:
        chunks.append((g, min(gp, NG - g)))
        g += chunks[-1][1]

    xa_v = xa[:].rearrange("p (G d) -> p G d", d=D)
    y_v = y[:].rearrange("p (G d) -> p G d", d=D)

    def split_dma(eng, dst, src, nsplit, pdim):
        if nsplit == 1:
            eng.dma_start(dst, src)
            return
        step = (pdim + nsplit - 1) // nsplit
        for s0 in range(0, pdim, step):
            s1 = min(s0 + step, pdim)
            eng.dma_start(dst[s0:s1], src[s0:s1])

    with tile.TileContext(nc) as tc:
        with (
            tc.tile_pool(name="wpool", bufs=1) as wpool,
            tc.tile_pool(name="apool", bufs=abufs) as apool,
            tc.tile_pool(name="opool", bufs=obufs) as opool,
            tc.tile_pool(name="psum", bufs=pbufs, space=bass.MemorySpace.PSUM) as psum,
        ):
            gper = (NG + wsplit - 1) // wsplit
            interleave_w = weng == "ginter"
            wt_tiles = []
            w_eng = nc.gpsimd if interleave_w else getattr(nc, weng)

            def load_w(wi):
                glo = wi * gper
                ghi = min(NG, (wi + 1) * gper)
                wtile = wpool.tile([KDIM, (ghi - glo) * MF], iodt, tag=f"w{wi}")
                w_eng.dma_start(wtile[:], wb[:, glo * MF : ghi * MF])
                wt_tiles.append(wtile)

            if not interleave_w:
                for wi in range(wsplit):
                    load_w(wi)

            def w_slice(g):
                wi, off = divmod(g, gper)
                return wt_tiles[wi][:, off * MF : (off + 1) * MF]

            k = 0
            pending = []  # delayed SWDGE output DMAs: (dst_ap, src_tile_ap)
            for ci, (gstart, cgp) in enumerate(chunks):
                if probe == "out_only":
                    o = opool.tile([MF, cgp, D], iodt, tag="o")
                    # cheap 1-elem writer so Tile orders the DMA after tile
                    # allocation; rest of the tile streams stale SBUF bytes
                    nc.vector.memset(o[:, :, :1], 0)
                    nc.gpsimd.dma_start(y_v[:, gstart : gstart + cgp], o[:])
                    continue
                a = apool.tile([KDIM, cgp, D], iodt, tag="a")
                split_dma(
                    in_engs[ci % len(in_engs)],
                    a[:],
                    xa_v[:, gstart : gstart + cgp],
                    in_split,
                    KDIM,
                )
                if probe == "in_only":
                    continue
                if interleave_w and ci < wsplit:
                    load_w(ci)
                if out_spart > 0 and len(pending) >= out_delay + out_flush:
                    for _ in range(out_flush):
                        dst, src = pending.pop(0)
                        nc.gpsimd.dma_start(dst, src)
                o = opool.tile([MF, cgp, D], iodt, tag="o")
                for gs in range(cgp):
                    gw = gstart + gs
                    for dcb in range(0, D // DCH, copy_banks):
                        p = psum.tile([MF, copy_banks * DCH], f32)
                        for j in range(copy_banks):
                            dc = dcb + j
                            nc.tensor.matmul(
                                p[:, j * DCH : (j + 1) * DCH],
                                lhsT=w_slice(gw),
                                rhs=a[:, gs, dc * DCH : (dc + 1) * DCH],
                                start=True,
                                stop=True,
                            )
                        dst = o[:, gs, dcb * DCH : (dcb + copy_banks) * DCH]
                        if k % 2 == 0:
                            nc.vector.tensor_copy(dst, p[:])
                        else:
                            nc.scalar.copy(dst, p[:])
                        k += 1
                y_dst = y_v[:, gstart : gstart + cgp]
                if out_spart > 0:
                    hp = MF - out_spart
                    if hp > 0:
                        nc.sync.dma_start(y_dst[:hp], o[:hp])
                    pending.append((y_dst[hp:], o[hp:]))
                elif out_hpart > 0:
                    nc.sync.dma_start(y_dst[:out_hpart], o[:out_hpart])
                    nc.gpsimd.dma_start(y_dst[out_hpart:], o[out_hpart:])
                else:
                    split_dma(
                        out_engs[ci % len(out_engs)],
                        y_dst,
                        o[:],
                        out_split,
                        MF,
                    )
            for dst, src in pending:
                nc.gpsimd.dma_start(dst, src)
    nc.compile()
    return nc


def _np_io_dtype(io_dtype):
    return {"bfloat16": BF16, "float16": np.float16, "float32": np.float32}[
        io_dtype
    ]


def prepare_in_maps(x, residual, post, comb, io_dtype="bfloat16"):
    """Host prepack: partition-major augmented data + block-diagonal weights,
    downcast to the I/O dtype. Padded tokens have zero weights -> zero output
    rows."""
    iodt = _np_io_dtype(io_dtype)
    x = np.asarray(x, dtype=np.float32)
    residual = np.asarray(residual, dtype=np.float32)
    post = np.asarray(post, dtype=np.float32)
    comb = np.asarray(comb, dtype=np.float32)

    xaug = np.zeros((TOKP, 5, D), iodt)
    xaug[:TOK, 0, :] = x.reshape(TOK, D)
    xaug[:TOK, 1:, :] = residual.reshape(TOK, M, D)

    caug = np.zeros((TOKP, 5, N), np.float32)
    caug[:TOK, 0, :] = post.reshape(TOK, N)
    caug[:TOK, 1:, :] = comb.reshape(TOK, M, N)

    ngt = TOKP // G  # total groups
    wall = np.zeros((ngt, KDIM, MF), np.float32)
    t = np.arange(G)
    rows = np.broadcast_to(
        5 * t[:, None, None] + np.arange(5)[None, :, None], (G, 5, N)
    ).ravel()
    cols = np.broadcast_to(
        N * t[:, None, None] + np.arange(N)[None, None, :], (G, 5, N)
    ).ravel()
    wall[:, rows, cols] = caug.reshape(ngt, G * 5 * N)

    in_maps = []
    for c in range(N_CORES):
        # [TPC, 5, D] -> [NG, G, 5, D] -> [G, 5, NG, D] -> [125, NG*D]
        xa_c = np.ascontiguousarray(
            xaug[c * TPC : (c + 1) * TPC]
            .reshape(NG, G, 5, D)
            .transpose(1, 2, 0, 3)
            .reshape(KDIM, NG * D)
        )
        wb_c = np.ascontiguousarray(
            wall[c * NG : (c + 1) * NG].transpose(1, 0, 2).reshape(KDIM, NG * MF)
        ).astype(iodt)
        in_maps.append({"xa": xa_c, "wb": wb_c})
    return in_maps


def unpack_y(results):
    """[100, NG*D] partition-major per core -> full (B, S, N, D) fp32."""
    ys = []
    for c in range(N_CORES):
        yc = np.asarray(results[c]["y"]).reshape(G, N, NG, D)
        ys.append(yc.transpose(2, 0, 1, 3).reshape(TPC, N, D))
    y = np.concatenate(ys, axis=0)[:TOK]
    return np.ascontiguousarray(y.reshape(B, S, N, D).astype(np.float32))


def kernel(x, residual, post, comb):
    global LAST_RESULTS, LAST_IN_MAPS
    in_maps = prepare_in_maps(
        x, residual, post, comb, BUILD_KWARGS.get("io_dtype", "bfloat16")
    )
    LAST_IN_MAPS = in_maps
    nc = _build_program(**BUILD_KWARGS)
    res = run_bass_kernel_spmd(nc, in_maps, list(range(N_CORES)))
    LAST_RESULTS = res
    return unpack_y(res.results)


# revision 18
# speedup vs baseline: 1.0784x; 1.0454x over previous
"""Trainium2 Bass kernel for nn_HcPost:

    out[b,s,n,d] = post[b,s,n] * x[b,s,d] + sum_m comb[b,s,m,n] * residual[b,s,m,d]

Strategy: per token this is a tiny K=5 contraction
    out[n,d] = sum_{m'} Caug[m',n] * Xaug[m',d]
with Xaug = [x; residual_0..3] and Caug = [post; comb_0..3].

We batch G=25 tokens into one TensorE matmul by building a block-diagonal
stationary weight matrix W[(t,m'), (t,n)] = Caug[t,m',n] (K=125, MF=100) on the
host, and streaming Xaug[(t,m'), d] as the moving operand. PSUM results
[(t,n), d] are evacuated to SBUF by VectorE/ScalarE and DMA'd out.

The kernel is HBM-bandwidth bound (~155 MB/core of fp32 traffic), so:
  - All HBM I/O is bf16 (inputs downcast on host, outputs upcast on host),
    halving traffic to ~78 MB/core. PSUM accumulation stays fp32; measured
    end-to-end relative error ~6e-3 (gate 2e-2).
  - HBM arrays are PARTITION-MAJOR ([p, group, d]) so each SDMA descriptor
    covers gp*4KB contiguous per partition instead of 4KB — descriptor
    fixed costs were ~30% of DMA busy time in the token-major layout.

Sharding: tokens (B*S = 16384) split evenly across 8 NeuronCores (data
parallel, no cross-core communication). Tokens are padded to 2050/core so each
core runs 82 uniform groups of 25.
"""

import sys

sys.path.insert(0, "/opt/trn_rl_repo")

import ml_dtypes
import numpy as np

import concourse.bass as bass
import concourse.mybir as mybir
import concourse.tile as tile
from concourse import bacc
from concourse.bass_utils import run_bass_kernel_spmd


def _ensure_ntff_hook():
    """Best-effort: register the axon NTFF profile hook so a BASS_TRACE=1 run
    can report exec_time_ns. The agent image's ``antenv`` lacks
    ``axon_hooks``; inject a minimal stand-in. No-op off-axon or on failure.
    """
    try:
        from concourse.bass_utils import axon_active

        if not axon_active():
            return
        try:
            from antenv.axon_hooks import get_axon_ntff_profile_hook  # noqa: F401

            return  # real module present
        except ImportError:
            pass
        import types

        import antenv
        from trn_agent_boot.trn_boot import _ntff_profile_via_ctypes

        mod = types.ModuleType("antenv.axon_hooks")
        mod._hook = _ntff_profile_via_ctypes("/opt/axon/libaxon_pjrt.so")
        mod.set_axon_ntff_profile_hook = lambda h: setattr(mod, "_hook", h)
        mod.get_axon_ntff_profile_hook = lambda: mod._hook
        sys.modules["antenv.axon_hooks"] = mod
        antenv.axon_hooks = mod
    except Exception:
        pass


_ensure_ntff_hook()

B, S, M, N, D = 4, 4096, 4, 4, 2048
TOK = B * S  # 16384 tokens
N_CORES = 8
G = 25  # tokens per PE group (contraction K = 5*G = 125 <= 128)
KDIM = 5 * G  # 125
MF = N * G  # 100 output partitions per group
TPC = 2050  # padded tokens per core (= 82 * 25)
NG = TPC // G  # 82 groups per core
TOKP = TPC * N_CORES  # 16400 padded tokens total
DCH = 512  # moving free-dim chunk (one PSUM bank)

BF16 = ml_dtypes.bfloat16

# Stashed BassKernelResults of the last kernel() call (for profiling in test
# harnesses via BASS_TRACE=1).
LAST_RESULTS = None
LAST_IN_MAPS = None

# Best configuration found on HW: all bulk DMAs via gpsimd SWDGE (spreads
# descriptors over all 16 SDMA engines; HWDGE pins reads to engines 0-4 /
# writes to 0-9), output DMAs delayed by several chunks so they never
# head-of-line block input DMAs in the gpsimd FIFO, weight slices interleaved
# into the first chunks. All HBM I/O bf16 + partition-major.
BUILD_KWARGS = dict(
    in_eng="gpsimd",
    gp=1,
    abufs=10,
    obufs=9,
    out_spart=100,
    out_delay=3,
    out_flush=1,
    wsplit=8,
    weng="pe",
    io_dtype="bfloat16",
)


def _build_program(in_eng="sync", out_eng="sync", in_split=1, out_split=1,
                   out_hpart=0, gp=4, abufs=2, obufs=2, pbufs=8,
                   out_spart=0, out_delay=4, out_flush=1, wsplit=1,
                   weng="sync", io_dtype="bfloat16", copy_banks=1,
                   probe=None):
    """Build the SPMD Bass program.

    HBM layouts (per core): xa [KDIM, NG*D] partition-major (xa[p, g*D+d] is
    (token g*25+p//5, m'=p%5, d) of the augmented input), wb [KDIM, NG*MF]
    block-diagonal weights, y [MF, NG*D] partition-major outputs.

    in_eng/out_eng: comma-separated engine cycle for input/output DMAs —
    elements from {sync, scalar, gpsimd}. in_split/out_split: issue each
    chunk's DMA as this many instructions (split along the partition dim).
    out_spart: if >0, that many rows of each output chunk go via delayed
    gpsimd SWDGE (delayed by out_delay chunks), the rest via sync HWDGE.
    io_dtype: dtype of all HBM tensors and SBUF tiles (PSUM stays fp32).
    """
    f32 = mybir.dt.float32
    iodt = getattr(mybir.dt, io_dtype)
    pe_w = weng == "pe"  # expand block-diag weights on-chip via PE+mask
    nc = bacc.Bacc(None, target_bir_lowering=False)
    xa = nc.dram_tensor("xa", [KDIM, NG * D], iodt, kind="ExternalInput")
    if pe_w:
        # compact weights [4, NG*125] + broadcast matmul rhs + 0/1 mask
        cwt = nc.dram_tensor("cwt", [N, NG * KDIM], iodt, kind="ExternalInput")
        emat = nc.dram_tensor("emat", [N, MF], iodt, kind="ExternalInput")
        mk = nc.dram_tensor("mk", [KDIM, MF], iodt, kind="ExternalInput")
    else:
        wb = nc.dram_tensor("wb", [KDIM, NG * MF], iodt, kind="ExternalInput")
    y = nc.dram_tensor("y", [MF, NG * D], iodt, kind="ExternalOutput")

    def engines(spec):
        return [getattr(nc, e) for e in spec.split(",")]

    in_engs = engines(in_eng)
    out_engs = engines(out_eng)

    chunks = []
    g = 0
    while g < NG:
        chunks.append((g, min(gp, NG - g)))
        g += chunks[-1][1]

    xa_v = xa[:].rearrange("p (G d) -> p G d", d=D)
    y_v = y[:].rearrange("p (G d) -> p G d", d=D)

    def split_dma(eng, dst, src, nsplit, pdim):
        if nsplit == 1:
            eng.dma_start(dst, src)
            return
        step = (pdim + nsplit - 1) // nsplit
        for s0 in range(0, pdim, step):
            s1 = min(s0 + step, pdim)
            eng.dma_start(dst[s0:s1], src[s0:s1])

    with tile.TileContext(nc) as tc:
        with (
            tc.tile_pool(name="wpool", bufs=1) as wpool,
            tc.tile_pool(name="wgen", bufs=3) as wgenpool,
            tc.tile_pool(name="apool", bufs=abufs) as apool,
            tc.tile_pool(name="opool", bufs=obufs) as opool,
            tc.tile_pool(
                name="psum",
                bufs=(6 if pe_w else pbufs),
                space=bass.MemorySpace.PSUM,
            ) as psum,
            tc.tile_pool(
                name="wpsum", bufs=2, space=bass.MemorySpace.PSUM
            ) as wpsum,
        ):
            gper = (NG + wsplit - 1) // wsplit
            interleave_w = weng == "ginter"
            wt_tiles = []

            if pe_w:
                cwt_t = wpool.tile([N, NG * KDIM], iodt, tag="cwt")
                nc.gpsimd.dma_start(cwt_t[:], cwt[:])
                emat_t = wpool.tile([N, MF], iodt, tag="emat")
                nc.gpsimd.dma_start(emat_t[:], emat[:])
                mk_t = wpool.tile([KDIM, MF], iodt, tag="mk")
                nc.gpsimd.dma_start(mk_t[:], mk[:])
                wgen = {}  # group -> generated bf16 weight tile in SBUF

                def gen_w(g):
                    # W_g = (cwt_g^T @ E) ⊙ mask : broadcast then zero
                    # off-diagonal blocks during the PSUM->SBUF eviction.
                    wp = wpsum.tile([KDIM, MF], f32)
                    nc.tensor.matmul(
                        wp[:],
                        lhsT=cwt_t[:, g * KDIM : (g + 1) * KDIM],
                        rhs=emat_t[:],
                        start=True,
                        stop=True,
                    )
                    wt = wgenpool.tile([KDIM, MF], iodt, tag="wg")
                    nc.vector.tensor_mul(wt[:], wp[:], mk_t[:])
                    wgen[g] = wt

                def w_slice(g):
                    return wgen.pop(g)[:]
            else:
                w_eng = nc.gpsimd if interleave_w else getattr(nc, weng)

                def load_w(wi):
                    glo = wi * gper
                    ghi = min(NG, (wi + 1) * gper)
                    wtile = wpool.tile(
                        [KDIM, (ghi - glo) * MF], iodt, tag=f"w{wi}"
                    )
                    w_eng.dma_start(wtile[:], wb[:, glo * MF : ghi * MF])
                    wt_tiles.append(wtile)

                if not interleave_w:
                    for wi in range(wsplit):
                        load_w(wi)

                def w_slice(g):
                    wi, off = divmod(g, gper)
                    return wt_tiles[wi][:, off * MF : (off + 1) * MF]

            k = 0
            pending = []  # delayed SWDGE output DMAs: (dst_ap, src_tile_ap)
            if pe_w:
                for g in range(chunks[0][0], chunks[0][0] + chunks[0][1]):
                    gen_w(g)
            for ci, (gstart, cgp) in enumerate(chunks):
                if probe == "out_only":
                    o = opool.tile([MF, cgp, D], iodt, tag="o")
                    # cheap 1-elem writer so Tile orders the DMA after tile
                    # allocation; rest of the tile streams stale SBUF bytes
                    nc.vector.memset(o[:, :, :1], 0)
                    nc.gpsimd.dma_start(y_v[:, gstart : gstart + cgp], o[:])
                    continue
                a = apool.tile([KDIM, cgp, D], iodt, tag="a")
                split_dma(
                    in_engs[ci % len(in_engs)],
                    a[:],
                    xa_v[:, gstart : gstart + cgp],
                    in_split,
                    KDIM,
                )
                if probe == "in_only":
                    continue
                if pe_w and ci + 1 < len(chunks):
                    ngs, ngn = chunks[ci + 1]
                    for g in range(ngs, ngs + ngn):
                        gen_w(g)
                if (not pe_w) and interleave_w and ci < wsplit:
                    load_w(ci)
                if out_spart > 0 and len(pending) >= out_delay + out_flush:
                    for _ in range(out_flush):
                        dst, src = pending.pop(0)
                        nc.gpsimd.dma_start(dst, src)
                o = opool.tile([MF, cgp, D], iodt, tag="o")
                for gs in range(cgp):
                    gw = gstart + gs
                    w_g = w_slice(gw)
                    for dcb in range(0, D // DCH, copy_banks):
                        p = psum.tile([MF, copy_banks * DCH], f32)
                        for j in range(copy_banks):
                            dc = dcb + j
                            nc.tensor.matmul(
                                p[:, j * DCH : (j + 1) * DCH],
                                lhsT=w_g,
                                rhs=a[:, gs, dc * DCH : (dc + 1) * DCH],
                                start=True,
                                stop=True,
                            )
                        dst = o[:, gs, dcb * DCH : (dcb + copy_banks) * DCH]
                        if k % 2 == 0:
                            nc.vector.tensor_copy(dst, p[:])
                        else:
                            nc.scalar.copy(dst, p[:])
                        k += 1
                y_dst = y_v[:, gstart : gstart + cgp]
                if out_spart > 0:
                    hp = MF - out_spart
                    if hp > 0:
                        nc.sync.dma_start(y_dst[:hp], o[:hp])
                    pending.append((y_dst[hp:], o[hp:]))
                elif out_hpart > 0:
                    nc.sync.dma_start(y_dst[:out_hpart], o[:out_hpart])
                    nc.gpsimd.dma_start(y_dst[out_hpart:], o[out_hpart:])
                else:
                    split_dma(
                        out_engs[ci % len(out_engs)],
                        y_dst,
                        o[:],
                        out_split,
                        MF,
                    )
            for dst, src in pending:
                nc.gpsimd.dma_start(dst, src)
    nc.compile()
    return nc


def _np_io_dtype(io_dtype):
    return {"bfloat16": BF16, "float16": np.float16, "float32": np.float32}[
        io_dtype
    ]


def prepare_in_maps(x, residual, post, comb, io_dtype="bfloat16"):
    """Host prepack: partition-major augmented data + block-diagonal weights,
    downcast to the I/O dtype. Padded tokens have zero weights -> zero output
    rows."""
    iodt = _np_io_dtype(io_dtype)
    x = np.asarray(x, dtype=np.float32)
    residual = np.asarray(residual, dtype=np.float32)
    post = np.asarray(post, dtype=np.float32)
    comb = np.asarray(comb, dtype=np.float32)

    xaug = np.zeros((TOKP, 5, D), iodt)
    xaug[:TOK, 0, :] = x.reshape(TOK, D)
    xaug[:TOK, 1:, :] = residual.reshape(TOK, M, D)

    caug = np.zeros((TOKP, 5, N), np.float32)
    caug[:TOK, 0, :] = post.reshape(TOK, N)
    caug[:TOK, 1:, :] = comb.reshape(TOK, M, N)

    ngt = TOKP // G  # total groups
    wall = np.zeros((ngt, KDIM, MF), np.float32)
    t = np.arange(G)
    rows = np.broadcast_to(
        5 * t[:, None, None] + np.arange(5)[None, :, None], (G, 5, N)
    ).ravel()
    cols = np.broadcast_to(
        N * t[:, None, None] + np.arange(N)[None, None, :], (G, 5, N)
    ).ravel()
    wall[:, rows, cols] = caug.reshape(ngt, G * 5 * N)

    # constants for the on-chip weight expansion path (weng="pe")
    emat = np.ascontiguousarray(np.tile(np.eye(N, dtype=np.float32), (1, G))).astype(iodt)
    mk = np.ascontiguousarray(
        np.kron(np.eye(G, dtype=np.float32), np.ones((5, N), np.float32))
    ).astype(iodt)

    in_maps = []
    for c in range(N_CORES):
        # [TPC, 5, D] -> [NG, G, 5, D] -> [G, 5, NG, D] -> [125, NG*D]
        xa_c = np.ascontiguousarray(
            xaug[c * TPC : (c + 1) * TPC]
            .reshape(NG, G, 5, D)
            .transpose(1, 2, 0, 3)
            .reshape(KDIM, NG * D)
        )
        wb_c = np.ascontiguousarray(
            wall[c * NG : (c + 1) * NG].transpose(1, 0, 2).reshape(KDIM, NG * MF)
        ).astype(iodt)
        # compact weights [4, NG*125]: cwt[n, g*125+5t+m'] = caug[g*25+t, m', n]
        cwt_c = np.ascontiguousarray(
            caug[c * TPC : (c + 1) * TPC]
            .reshape(NG, G, 5, N)
            .transpose(3, 0, 1, 2)
            .reshape(N, NG * KDIM)
        ).astype(iodt)
        in_maps.append(
            {"xa": xa_c, "wb": wb_c, "cwt": cwt_c, "emat": emat, "mk": mk}
        )
    return in_maps


def unpack_y(results):
    """[100, NG*D] partition-major per core -> full (B, S, N, D) fp32."""
    ys = []
    for c in range(N_CORES):
        yc = np.asarray(results[c]["y"]).reshape(G, N, NG, D)
        ys.append(yc.transpose(2, 0, 1, 3).reshape(TPC, N, D))
    y = np.concatenate(ys, axis=0)[:TOK]
    return np.ascontiguousarray(y.reshape(B, S, N, D).astype(np.float32))


def kernel(x, residual, post, comb):
    global LAST_RESULTS, LAST_IN_MAPS
    in_maps = prepare_in_maps(
        x, residual, post, comb, BUILD_KWARGS.get("io_dtype", "bfloat16")
    )
    LAST_IN_MAPS = in_maps
    nc = _build_program(**BUILD_KWARGS)
    res = run_bass_kernel_spmd(nc, in_maps, list(range(N_CORES)))
    LAST_RESULTS = res
    return unpack_y(res.results)
